# revision 5
# baseline (speedup 1.0000x reference)
"""Trainium2 Bass kernel for nn_BinaryMemory (retrieval_knn).

reference:
    gated = sigmoid(query @ W.T + b)                      # [1, D], D=4096
    sims  = 1 - mean(|memory - gated|, axis=-1)           # [N],   N=16384
    mask  = sims >= 0.8

Sharding (8 cores, no collectives): shard the D axis; core c owns
d-chunk [c*512, (c+1)*512). All bulk tensors stream as fp8_e3m4.
Layout is d-on-partitions (memory shard transposed host-side to
[512 d, 16384 n]) so the gate value g[d] is a per-partition scalar.

|m-g| split: DVE pieces compute min(m-g,0) via one fused
tensor_scalar(sub, min); the m-term sum rides on the PE (ones^T @ m,
gate-independent) and the g-term is corrected on host per (k, group).
ScalarE pieces compute |m-g| in one op via activation(Abs, scale=-1,
bias=+g). Reductions over d run on the PE into psum rows at quadrant
offsets (4-way tile_position concurrency).

v3 structure (from v1/v2 traces): the whole kernel is paced by the
single HWDGE stream (sync ring) whose per-DMA completion semaphores
lag the bytes by an HBM-receipt latency that ramps ~0.3 -> ~3us under
load, and by the two elementwise engines whose combined work nearly
fills the stream window. Changes that matter:
 - W is repacked c-major and shipped as 4 chunk-blocks interleaved
   with the first n-tiles (ring order c8, W0, t00ab, W1, t01, W2, W3,
   t02, t03, rest) and the gate runs PER CHUNK (32 matmuls + fused
   strip-sum/transpose matmul + Sigmoid-with-bias straight from psum),
   so g[chunk0] is ready ~12.5us and the elementwise engines start
   ~8us earlier than with the monolithic gate.
 - No SWDGE anywhere (v1's gpsimd out-DMAs skewed SDMA engine 15 from
   t0: +1.3us late start, -10% rate, ~3us added to every completion
   late in the stream) and no HWDGE gen on the ScalarE queue (v2: each
   out-DMA gen stole ~0.6us of ABS throughput). All outputs ride the
   sync ring, emitted after the last input dma_start.
 - First/last tiles split into half-DMAs so the receipt latency hides
   behind half-sized elementwise ops at both ends of the stream.
 - The gate-sum correction runs right after the gate, not at the end.
"""
import sys

sys.path.insert(0, "/opt/trn_rl_repo")

import numpy as np
import ml_dtypes

import concourse.bacc as bacc
import concourse.mybir as mybir
import concourse.tile as tile
from concourse.bass_utils import run_bass_kernel_spmd

N_CORES = 8
D = 4096
N = 16384
D_SH = D // N_CORES            # 512 dims per core
DC = D_SH // 128               # 4 d-chunks (partition blocks)
NT = 4096                      # n per tile
NK = N // NT                   # 4 n-chunks
NG = NT // 512                 # 8 psum groups per tile
THRESHOLD = 0.8

# ScalarE (Abs) full tiles; everything else takes the DVE min path
_FULL_A = {(0, 1), (0, 3), (1, 2), (2, 0), (2, 2), (3, 0)}


def _pieces(k, c):
    """(k, c) -> list of (n_lo, n_hi, engine) covering [0, NT)."""
    if (k, c) == (0, 0):
        return [(0, 2048, "V"), (2048, 4096, "V")]
    if (k, c) == (3, 2):
        return [(0, 2048, "A"), (2048, 4096, "V")]
    if (k, c) == (3, 3):
        return [(0, 2048, "V"), (2048, 4096, "V")]
    return [(0, 4096, "A" if (k, c) in _FULL_A else "V")]


def _dve_chunks(k, j):
    """chunks whose piece covering group j of k-block is DVE (min path)."""
    out = []
    for c in range(DC):
        for lo, hi, eng in _pieces(k, c):
            if lo <= j * 512 < hi and eng == "V":
                out.append(c)
    return out


_CACHE = {}


def _build():
    f32 = mybir.dt.float32
    f16 = mybir.dt.float16
    f8 = mybir.dt.float8e3
    A = mybir.AluOpType
    AF = mybir.ActivationFunctionType
    nc = bacc.Bacc(
        "TRN2", target_bir_lowering=False, debug=False, num_devices=N_CORES
    )

    memT = nc.dram_tensor("memT", [D_SH, N], f8, kind="ExternalInput")
    # W shard, host-packed c-major:
    # wtpc[p, c*4096 + j*128 + n'] = W[cglobal*128 + n', j*128 + p]
    wtpc = nc.dram_tensor("wtpc", [128, 32 * D_SH], f8, kind="ExternalInput")
    # packed fp8 constants: cols 0:32 qcol, 32 ones, 33 neg2, 34 selsum
    c8 = nc.dram_tensor("c8", [128, 35], f8, kind="ExternalInput")
    # packed f32 constants: cols 0:4 b columns, 4 ones
    c32 = nc.dram_tensor("c32", [128, 5], f32, kind="ExternalInput")
    outp = nc.dram_tensor("outp", [33, 512], f32, kind="ExternalOutput")

    with tile.TileContext(nc) as tc:
        with (
            tc.tile_pool(name="wts", bufs=1) as wpool,
            tc.tile_pool(name="mem", bufs=8) as mpool,
            tc.tile_pool(name="memh", bufs=4) as mhpool,
            tc.tile_pool(name="dts", bufs=3) as dpool,
            tc.tile_pool(name="dtsh", bufs=3) as dhpool,
            tc.tile_pool(name="acts", bufs=2) as apool,
            tc.tile_pool(name="acth", bufs=1) as ahpool,
            tc.tile_pool(name="cp", bufs=8) as cppool,
            tc.tile_pool(name="small", bufs=1) as spool,
            tc.tile_pool(name="psz", bufs=1, space="PSUM") as pzpool,
            tc.tile_pool(name="pst", bufs=1, space="PSUM") as ptpool,
            tc.tile_pool(name="psb", bufs=6, space="PSUM") as pspool,
        ):
            # ---- constants first on the sync ring ----
            c8_sb = spool.tile([128, 35], f8, tag="c8")
            nc.sync.dma_start(out=c8_sb[:], in_=c8[:])
            c32_sb = spool.tile([128, 5], f32, tag="c32")
            nc.scalar.dma_start(out=c32_sb[:], in_=c32[:])
            qc_sb = c8_sb[:, 0:32]
            ones_sb = c8_sb[:, 32:33]
            neg2_sb = c8_sb[:, 33:34]
            selsum8 = c8_sb[:, 34:35]
            b4 = c32_sb[:, 0:4]
            ones32_sb = c32_sb[:, 4:5]
            # activation table preload with no DMA dependency
            dum_in = spool.tile([1, 4], f32, tag="dumin")
            nc.vector.memset(dum_in[:], 0.25)
            dum = spool.tile([1, 4], f32, tag="dum")
            nc.scalar.activation(dum[:], dum_in[:], AF.Sigmoid)
            nc.scalar.activation(dum[:], dum_in[:], AF.Abs)

            zps = pzpool.tile([128, D_SH], f32, tag="z")
            nc.vector.memset(zps[:], 0.0)
            ztp = ptpool.tile([128, 8], f32, tag="ztp")
            zcp = spool.tile([128, D_SH], f16, tag="zcp")
            gpos = spool.tile([128, DC], f32, tag="gpos")

            out_dmas = []   # (outp_slice, sbuf_ap) deferred to post-loop

            mem_dma = {}    # (k,c,lo) -> sbuf tile

            def emit_mem_dma(k, c, lo, hi):
                w = hi - lo
                pool = mpool if w == NT else mhpool
                mt = pool.tile([128, w], f8, tag="m" if w == NT else "mh")
                nc.sync.dma_start(
                    out=mt[:],
                    in_=memT[c * 128 : (c + 1) * 128, k * NT + lo : k * NT + hi],
                )
                mem_dma[(k, c, lo)] = mt
                return mt

            def emit_mterm(k, c, lo, hi, banks, seen, total):
                mt = mem_dma[(k, c, lo)]
                for j in range(lo // 512, hi // 512):
                    s = slice(j * 512 - lo, j * 512 - lo + 512)
                    nc.tensor.matmul(
                        banks[j // 4][32 * (j % 4) : 32 * (j % 4) + 1, :],
                        ones_sb,
                        mt[:, s],
                        start=(seen[j] == 0),
                        stop=(seen[j] == total[j] - 1),
                        tile_position=(0, 32 * (j % 4)),
                        skip_group_check=True,
                    )
                    seen[j] += 1

            wts = []

            def emit_wblock(c):
                wt_sb = wpool.tile([128, 32 * 128], f8, tag=f"wt{c}")
                nc.sync.dma_start(
                    out=wt_sb[:],
                    in_=wtpc[:, c * NT : (c + 1) * NT],
                )
                wts.append(wt_sb)

            def emit_gate_chunk(c):
                # 32 matmuls: stationary = q column j, moving = W block
                # slice; quadrant-cycled rows for 4-way concurrency
                for j in range(32):
                    r = j % 4
                    nc.tensor.matmul(
                        zps[32 * r : 32 * r + 1, c * 128 : (c + 1) * 128],
                        qc_sb[:, j : j + 1],
                        wts[c][:, j * 128 : (j + 1) * 128],
                        start=(j < 4),
                        stop=(j >= 28),
                        tile_position=(0, 32 * r),
                        skip_group_check=True,
                    )
                with tc.high_priority():
                    # psum -> fp16 (cast3 rides ScalarE: DVE is busy with
                    # tiles by then and ScalarE idles between sigmoids)
                    csl = slice(c * 128, (c + 1) * 128)
                    if c == 3:
                        nc.scalar.activation(zcp[:, csl], zps[:, csl], AF.Copy)
                    else:
                        nc.vector.tensor_copy(zcp[:, csl], zps[:, csl])
                    # fused strip-sum + transpose
                    nc.tensor.matmul(
                        ztp[:, c : c + 1],
                        zcp[:, csl],
                        selsum8,
                        start=True,
                        stop=True,
                        skip_group_check=True,
                    )
                    # g_c = sigmoid(z_c + b_c) straight from psum
                    nc.scalar.activation(
                        gpos[:, c : c + 1], ztp[:, c : c + 1], AF.Sigmoid,
                        bias=b4[:, c : c + 1],
                    )

            def emit_elem(k, c, lo, hi, eng, banks, seen, total):
                mt = mem_dma[(k, c, lo)]
                w = hi - lo
                if eng == "A":
                    pool = apool if w == NT else ahpool
                    at = pool.tile([128, w], f8, tag="a" if w == NT else "ah")
                    nc.scalar.activation(
                        at[:], mt[:], AF.Abs,
                        bias=gpos[:, c : c + 1], scale=-1.0,
                    )
                    src_, stat = at, ones_sb
                else:
                    pool = dpool if w == NT else dhpool
                    dt = pool.tile([128, w], f8, tag="d" if w == NT else "dh")
                    nc.vector.tensor_scalar(
                        dt[:], mt[:],
                        gpos[:, c : c + 1], 0.0,
                        A.subtract, A.min,
                    )
                    src_, stat = dt, neg2_sb
                for j in range(lo // 512, hi // 512):
                    s = slice(j * 512 - lo, j * 512 - lo + 512)
                    nc.tensor.matmul(
                        banks[j // 4][32 * (j % 4) : 32 * (j % 4) + 1, :],
                        stat,
                        src_[:, s],
                        start=(seen[j] == 0),
                        stop=(seen[j] == total[j] - 1),
                        tile_position=(0, 32 * (j % 4)),
                        skip_group_check=True,
                    )
                    seen[j] += 1

            def bank_counts(k):
                total = [0] * NG
                for c in range(DC):
                    for lo, hi, eng in _pieces(k, c):
                        npass = 1 if eng == "A" else 2
                        for j in range(lo // 512, hi // 512):
                            total[j] += npass
                return total

            # ---- k=0 (interleaved with W blocks on the ring) ----
            banks0 = [
                pspool.tile([128, 512], f32, tag="bank", name="bank0a"),
                pspool.tile([128, 512], f32, tag="bank", name="bank0b"),
            ]
            total0 = bank_counts(0)
            seen0 = [0] * NG

            emit_wblock(0)
            emit_mem_dma(0, 0, 0, 2048)
            emit_mem_dma(0, 0, 2048, 4096)
            emit_gate_chunk(0)
            emit_mterm(0, 0, 0, 2048, banks0, seen0, total0)
            emit_mterm(0, 0, 2048, 4096, banks0, seen0, total0)
            emit_wblock(1)
            emit_mem_dma(0, 1, 0, 4096)
            emit_gate_chunk(1)
            emit_wblock(2)
            emit_wblock(3)
            emit_mem_dma(0, 2, 0, 4096)
            emit_mem_dma(0, 3, 0, 4096)
            emit_gate_chunk(2)
            emit_gate_chunk(3)
            emit_mterm(0, 2, 0, 4096, banks0, seen0, total0)

            # gate-sum correction, right after the gate
            with tc.high_priority():
                gsps = ztp[0:1, 4:8]
                nc.tensor.matmul(
                    gsps, ones32_sb, gpos[:], start=True, stop=True,
                    skip_group_check=True,
                )
                gs_sb = spool.tile([1, DC], f32, tag="gs")
                nc.scalar.activation(gs_sb[:], gsps, AF.Copy)

            # k=0 elementwise
            emit_elem(0, 0, 0, 2048, "V", banks0, seen0, total0)
            emit_elem(0, 0, 2048, 4096, "V", banks0, seen0, total0)
            emit_elem(0, 1, 0, 4096, "A", banks0, seen0, total0)
            emit_elem(0, 2, 0, 4096, "V", banks0, seen0, total0)
            emit_elem(0, 3, 0, 4096, "A", banks0, seen0, total0)

            def emit_copies(k, banks):
                cp = cppool.tile([128, 1024], f32, tag="cp")
                for h in range(2):
                    csl = slice(512 * h, 512 * h + 512)
                    if (k, h) == (3, 0):
                        nc.scalar.activation(cp[:, csl], banks[h][:], AF.Copy)
                    else:
                        nc.vector.tensor_copy(cp[:, csl], banks[h][:])
                    out_dmas.append(
                        (outp[8 * k + 4 * h : 8 * k + 4 * h + 4, :],
                         cp[0:128:32, csl])
                    )

            emit_copies(0, banks0)

            # ---- k=1..3 ----
            for k in range(1, NK):
                banks = [
                    pspool.tile([128, 512], f32, tag="bank", name=f"bank{k}a"),
                    pspool.tile([128, 512], f32, tag="bank", name=f"bank{k}b"),
                ]
                total = bank_counts(k)
                seen = [0] * NG
                plist = [
                    (c, lo, hi, eng)
                    for c in range(DC)
                    for lo, hi, eng in _pieces(k, c)
                ]
                for c, lo, hi, eng in plist:
                    emit_mem_dma(k, c, lo, hi)
                    if eng == "V":
                        emit_mterm(k, c, lo, hi, banks, seen, total)
                for c, lo, hi, eng in plist:
                    emit_elem(k, c, lo, hi, eng, banks, seen, total)
                emit_copies(k, banks)

            # ---- outputs: sync ring, after every input dma_start ----
            nc.sync.dma_start(out=outp[32:33, 0:DC], in_=gs_sb[:])
            for dst, src in out_dmas:
                nc.sync.dma_start(out=dst, in_=src)

    nc.compile()
    return nc


def _get_nc():
    if "nc" not in _CACHE:
        _CACHE["nc"] = _build()
    return _CACHE["nc"]


def kernel(query, W, b, memory, _trace=False, _return_raw=False):
    f8 = ml_dtypes.float8_e3m4
    query = np.asarray(query, dtype=np.float32)
    W = np.asarray(W, dtype=np.float32)
    b = np.asarray(b, dtype=np.float32)
    memory = np.asarray(memory, dtype=np.float32)

    mem8T = np.ascontiguousarray(memory.astype(f8).T)       # [D, N] fp8
    W8 = W.astype(f8)
    q8 = query.reshape(32, 128).astype(f8).T                # [128, 32]
    c8 = np.zeros((128, 35), dtype=f8)
    c8[:, 0:32] = q8
    c8[:, 32] = f8(1.0)
    c8[:, 33] = f8(-2.0)
    c8[0:128:32, 34] = f8(1.0)

    in_maps = []
    for c in range(N_CORES):
        sl = slice(c * D_SH, (c + 1) * D_SH)
        wsh = W8[sl, :]                       # [512, 4096]
        # wtpc[p, cc*4096 + j*128 + n'] = wsh[cc*128 + n', j*128 + p]
        wtpc = np.ascontiguousarray(
            wsh.reshape(4, 128, 32, 128).transpose(3, 0, 2, 1).reshape(128, -1)
        )
        c32 = np.zeros((128, 5), dtype=np.float32)
        c32[:, 0:4] = b[sl].reshape(4, 128).T
        c32[:, 4] = 1.0
        in_maps.append(
            {
                "memT": np.ascontiguousarray(mem8T[sl, :]),
                "wtpc": wtpc,
                "c8": c8,
                "c32": c32,
            }
        )

    nc = _get_nc()
    res = run_bass_kernel_spmd(
        nc, in_maps, list(range(N_CORES)), trace=_trace
    )

    total = np.zeros(N, dtype=np.float64)
    for c in range(N_CORES):
        out = res.results[c]["outp"]
        gsum = out[32, 0:DC].astype(np.float64)   # sum of g per d-chunk
        rows = out[0:32].reshape(NK, NG, 512)
        corr = np.array(
            [
                [sum(gsum[ci] for ci in _dve_chunks(k, j)) for j in range(NG)]
                for k in range(NK)
            ]
        )
        total += (rows - corr[:, :, None]).reshape(N)
    sims = (1.0 - total / D).astype(np.float32)
    mask = sims >= THRESHOLD
    if _return_raw:
        return (sims, mask), res
    return sims, mask


# revision 8
# speedup vs baseline: 1.2670x; 1.2670x over previous
"""Trainium2 Bass kernel for nn_BinaryMemory (retrieval_knn).

reference:
    gated = sigmoid(query @ W.T + b)                      # [1, D], D=4096
    sims  = 1 - mean(|memory - gated|, axis=-1)           # [N],   N=16384
    mask  = sims >= 0.8

Sharding (8 cores, no collectives): shard the D axis; core c owns
d-chunk [c*512, (c+1)*512). All bulk tensors stream as fp8_e3m4.
Layout is d-on-partitions (memory shard transposed host-side to
[512 d, 16384 n]) so the gate value g[d] is a per-partition scalar.

|m-g| split: DVE pieces compute min(m-g,0) via one fused
tensor_scalar(sub, min); the m-term sum rides on the PE (ones^T @ m,
gate-independent) and the g-term is corrected on host per (k, group).
ScalarE pieces compute |m-g| in one op via activation(Abs, scale=-1,
bias=+g). Reductions over d run on the PE into psum rows at quadrant
offsets (4-way tile_position concurrency).

v5 design, driven by traces of 4 prior variants:
 - The single HWDGE stream delivers COMPLETIONS as a FIFO conveyor at
   ~1.25us per DMA nearly independent of size (HBM-receipt pipeline),
   so DMA COUNT is the currency: W ships as ONE 2MB DMA, the 16 memory
   tiles are unsplit, and outputs are 5 DMAs ([4,1024] fp16 per k + the
   gate-sum row). No SWDGE anywhere (it skews SDMA engine 15 ~10%
   slower from t0); no DMA gen on the ScalarE queue (it costs ABS
   throughput); outputs ride the sync ring after the last input gen.
 - The gate runs per 128-d chunk (32 matmuls each, quadrant-cycled;
   fused strip-sum/transpose matmul; Sigmoid straight off psum with
   per-partition bias b), with the PE HAM-warmed by junk matmuls so
   g[c] lands at ~0.9us intervals right behind the single W completion
   - just ahead of the first four tiles' completions. Elementwise
   starts ~5us earlier than with the monolithic gate.
 - psum->SBUF bank copies are fp16 (2x engine rate, half the out-DMA
   bytes); the host upcasts.
"""
import sys

sys.path.insert(0, "/opt/trn_rl_repo")

import numpy as np
import ml_dtypes

import concourse.bacc as bacc
import concourse.mybir as mybir
import concourse.tile as tile
from concourse.bass_utils import run_bass_kernel_spmd

N_CORES = 8
D = 4096
N = 16384
D_SH = D // N_CORES            # 512 dims per core
DC = D_SH // 128               # 4 d-chunks (partition blocks)
NT = 4096                      # n per tile
NK = N // NT                   # 4 n-chunks
NG = NT // 512                 # 8 psum groups per tile
THRESHOLD = 0.8

# ScalarE (Abs) tiles; everything else takes the DVE min path
_ACT_TILES = {(0, 3), (1, 2), (2, 0), (2, 2), (3, 0)}


def _dve_chunks(k, j):
    return [c for c in range(DC) if (k, c) not in _ACT_TILES]


_CACHE = {}


def _build():
    f32 = mybir.dt.float32
    f16 = mybir.dt.float16
    f8 = mybir.dt.float8e3
    A = mybir.AluOpType
    AF = mybir.ActivationFunctionType
    nc = bacc.Bacc(
        "TRN2", target_bir_lowering=False, debug=False, num_devices=N_CORES
    )

    memT = nc.dram_tensor("memT", [D_SH, N], f8, kind="ExternalInput")
    # W shard, host-packed c-major:
    # wtpc[p, c*4096 + j*128 + n'] = W[cglobal*128 + n', j*128 + p]
    wtpc = nc.dram_tensor("wtpc", [128, 32 * D_SH], f8, kind="ExternalInput")
    # packed fp8 constants: cols 0:32 qcol, 32 ones, 33 neg2, 34 selsum
    c8 = nc.dram_tensor("c8", [128, 35], f8, kind="ExternalInput")
    # packed f32 constants: cols 0:4 b columns, 4 ones
    c32 = nc.dram_tensor("c32", [128, 5], f32, kind="ExternalInput")
    # rows 4k+i: group j=4h+i of block k at cols [512h:512h+512]; row 16
    # cols 0:4 = per-chunk gate sums
    outp = nc.dram_tensor("outp", [17, 1024], f16, kind="ExternalOutput")

    with tile.TileContext(nc) as tc:
        with (
            tc.tile_pool(name="wts", bufs=1) as wpool,
            tc.tile_pool(name="mem", bufs=11) as mpool,
            tc.tile_pool(name="dts", bufs=3) as dpool,
            tc.tile_pool(name="acts", bufs=2) as apool,
            tc.tile_pool(name="cp", bufs=4) as cppool,
            tc.tile_pool(name="small", bufs=1) as spool,
            tc.tile_pool(name="psz", bufs=1, space="PSUM") as pzpool,
            tc.tile_pool(name="pst", bufs=1, space="PSUM") as ptpool,
            tc.tile_pool(name="psb", bufs=6, space="PSUM") as pspool,
        ):
            # ---- constants first, then the single W DMA ----
            c8_sb = spool.tile([128, 35], f8, tag="c8")
            nc.sync.dma_start(out=c8_sb[:], in_=c8[:])
            wt_sb = wpool.tile([128, 32 * D_SH], f8, tag="wt")
            nc.sync.dma_start(out=wt_sb[:], in_=wtpc[:])
            c32_sb = spool.tile([128, 5], f32, tag="c32")
            nc.scalar.dma_start(out=c32_sb[:], in_=c32[:])
            qc_sb = c8_sb[:, 0:32]
            ones_sb = c8_sb[:, 32:33]
            neg2_sb = c8_sb[:, 33:34]
            selsum8 = c8_sb[:, 34:35]
            b4 = c32_sb[:, 0:4]
            ones32_sb = c32_sb[:, 4:5]
            # activation table preload with no DMA dependency
            dum_in = spool.tile([1, 4], f32, tag="dumin")
            nc.vector.memset(dum_in[:], 0.25)
            dum = spool.tile([1, 4], f32, tag="dum")
            nc.scalar.activation(dum[:], dum_in[:], AF.Sigmoid)
            nc.scalar.activation(dum[:], dum_in[:], AF.Abs)

            zps = pzpool.tile([128, D_SH], f32, tag="z")
            nc.vector.memset(zps[:], 0.0)
            ztp = ptpool.tile([128, 512], f32, tag="ztp")
            zcp = spool.tile([128, D_SH], f16, tag="zcp")
            gpos = spool.tile([128, DC], f32, tag="gpos")

            # PE warmup: junk matmuls (zeroed SBUF, psum region reused by
            # strip-sums whose start=True clears it) keep HAM at K=8/8
            # through the W-completion so the gate runs at 2.4GHz.
            wjunk = spool.tile([128, 512], f8, tag="wjunk")
            nc.vector.memset(wjunk[:], 0.5)
            for _w in range(14):
                nc.tensor.matmul(
                    ztp[0:1, :], wjunk[:, 0:1], wjunk[:, 0:512],
                    start=(_w == 0), stop=(_w == 13), skip_group_check=True,
                )

            # ---- gate, per 128-d chunk ----
            for c in range(DC):
                for j in range(32):
                    r = j % 4
                    nc.tensor.matmul(
                        zps[32 * r : 32 * r + 1, c * 128 : (c + 1) * 128],
                        qc_sb[:, j : j + 1],
                        wt_sb[:, c * NT + j * 128 : c * NT + (j + 1) * 128],
                        start=(j < 4),
                        stop=(j >= 28),
                        tile_position=(0, 32 * r),
                        skip_group_check=True,
                    )
                with tc.high_priority():
                    csl = slice(c * 128, (c + 1) * 128)
                    nc.scalar.activation(zcp[:, csl], zps[:, csl], AF.Copy)
                    # fused strip-sum + transpose
                    nc.tensor.matmul(
                        ztp[:, c : c + 1],
                        zcp[:, csl],
                        selsum8,
                        start=True,
                        stop=True,
                        skip_group_check=True,
                    )
                    # g_c = sigmoid(z_c + b_c) straight from psum
                    nc.scalar.activation(
                        gpos[:, c : c + 1], ztp[:, c : c + 1], AF.Sigmoid,
                        bias=b4[:, c : c + 1],
                    )

            # gate-sum correction, right after the gate
            with tc.high_priority():
                gsps = ztp[0:1, 4:8]
                nc.tensor.matmul(
                    gsps, ones32_sb, gpos[:], start=True, stop=True,
                    skip_group_check=True,
                )
                gs_sb = spool.tile([1, DC], f16, tag="gs")
                nc.scalar.activation(gs_sb[:], gsps, AF.Copy)

            out_dmas = [(outp[16:17, 0:DC], gs_sb[:])]

            # ---- main loop ----
            for k in range(NK):
                banks = [
                    pspool.tile([128, 512], f32, tag="bank", name=f"bank{k}a"),
                    pspool.tile([128, 512], f32, tag="bank", name=f"bank{k}b"),
                ]
                total = [0] * NG
                for c in range(DC):
                    npass = 1 if (k, c) in _ACT_TILES else 2
                    for j in range(NG):
                        total[j] += npass
                seen = [0] * NG
                mts = []
                for c in range(DC):
                    mt = mpool.tile([128, NT], f8, tag="m")
                    nc.sync.dma_start(
                        out=mt[:],
                        in_=memT[c * 128 : (c + 1) * 128, k * NT : (k + 1) * NT],
                    )
                    mts.append(mt)
                    if (k, c) not in _ACT_TILES:
                        for j in range(NG):
                            nc.tensor.matmul(
                                banks[j // 4][32 * (j % 4) : 32 * (j % 4) + 1, :],
                                ones_sb,
                                mt[:, j * 512 : (j + 1) * 512],
                                start=(seen[j] == 0),
                                stop=(seen[j] == total[j] - 1),
                                tile_position=(0, 32 * (j % 4)),
                                skip_group_check=True,
                            )
                            seen[j] += 1
                for c in range(DC):
                    mt = mts[c]
                    if (k, c) in _ACT_TILES:
                        at = apool.tile([128, NT], f8, tag="a")
                        nc.scalar.activation(
                            at[:], mt[:], AF.Abs,
                            bias=gpos[:, c : c + 1], scale=-1.0,
                        )
                        src_, stat = at, ones_sb
                    else:
                        dt = dpool.tile([128, NT], f8, tag="d")
                        nc.vector.tensor_scalar(
                            dt[:], mt[:],
                            gpos[:, c : c + 1], 0.0,
                            A.subtract, A.min,
                        )
                        src_, stat = dt, neg2_sb
                    for j in range(NG):
                        nc.tensor.matmul(
                            banks[j // 4][32 * (j % 4) : 32 * (j % 4) + 1, :],
                            stat,
                            src_[:, j * 512 : (j + 1) * 512],
                            start=(seen[j] == 0),
                            stop=(seen[j] == total[j] - 1),
                            tile_position=(0, 32 * (j % 4)),
                            skip_group_check=True,
                        )
                        seen[j] += 1
                cp = cppool.tile([128, 1024], f16, tag="cp")
                for h in range(2):
                    csl = slice(512 * h, 512 * h + 512)
                    if h == 0:
                        nc.scalar.activation(cp[:, csl], banks[h][:], AF.Copy)
                    else:
                        nc.vector.tensor_copy(cp[:, csl], banks[h][:])
                out_dmas.append((outp[4 * k : 4 * k + 4, :], cp[0:128:32, :]))

            # ---- outputs: sync ring, after every input dma_start ----
            for dst, src in out_dmas:
                nc.sync.dma_start(out=dst, in_=src)

    nc.compile()
    return nc


def _get_nc():
    if "nc" not in _CACHE:
        _CACHE["nc"] = _build()
    return _CACHE["nc"]


def kernel(query, W, b, memory, _trace=False, _return_raw=False):
    f8 = ml_dtypes.float8_e3m4
    query = np.asarray(query, dtype=np.float32)
    W = np.asarray(W, dtype=np.float32)
    b = np.asarray(b, dtype=np.float32)
    memory = np.asarray(memory, dtype=np.float32)

    mem8T = np.ascontiguousarray(memory.astype(f8).T)       # [D, N] fp8
    W8 = W.astype(f8)
    q8 = query.reshape(32, 128).astype(f8).T                # [128, 32]
    c8 = np.zeros((128, 35), dtype=f8)
    c8[:, 0:32] = q8
    c8[:, 32] = f8(1.0)
    c8[:, 33] = f8(-2.0)
    c8[0:128:32, 34] = f8(1.0)

    in_maps = []
    for c in range(N_CORES):
        sl = slice(c * D_SH, (c + 1) * D_SH)
        wsh = W8[sl, :]                       # [512, 4096]
        # wtpc[p, cc*4096 + j*128 + n'] = wsh[cc*128 + n', j*128 + p]
        wtpc = np.ascontiguousarray(
            wsh.reshape(4, 128, 32, 128).transpose(3, 0, 2, 1).reshape(128, -1)
        )
        c32 = np.zeros((128, 5), dtype=np.float32)
        c32[:, 0:4] = b[sl].reshape(4, 128).T
        c32[:, 4] = 1.0
        in_maps.append(
            {
                "memT": np.ascontiguousarray(mem8T[sl, :]),
                "wtpc": wtpc,
                "c8": c8,
                "c32": c32,
            }
        )

    nc = _get_nc()
    res = run_bass_kernel_spmd(
        nc, in_maps, list(range(N_CORES)), trace=_trace
    )

    total = np.zeros(N, dtype=np.float64)
    for c in range(N_CORES):
        out = res.results[c]["outp"].astype(np.float64)
        gsum = out[16, 0:DC]                  # sum of g per d-chunk
        # row 4k+i, col 512h+n  ->  block k, group j=4h+i
        rows = out[0:16].reshape(NK, 4, 2, 512).transpose(0, 2, 1, 3)
        rows = np.ascontiguousarray(rows).reshape(NK, NG, 512)
        corr = np.array(
            [
                [sum(gsum[ci] for ci in _dve_chunks(k, j)) for j in range(NG)]
                for k in range(NK)
            ]
        )
        total += (rows - corr[:, :, None]).reshape(N)
    sims = (1.0 - total / D).astype(np.float32)
    mask = sims >= THRESHOLD
    if _return_raw:
        return (sims, mask), res
    return sims, mask


# revision 9
# speedup vs baseline: 1.3086x; 1.0328x over previous
"""Trainium2 Bass kernel for nn_BinaryMemory (retrieval_knn).

reference:
    gated = sigmoid(query @ W.T + b)                      # [1, D], D=4096
    sims  = 1 - mean(|memory - gated|, axis=-1)           # [N],   N=16384
    mask  = sims >= 0.8

Sharding (8 cores, no collectives): shard the D axis; core c owns
d-chunk [c*512, (c+1)*512). All bulk tensors stream as fp8_e3m4.
Layout is d-on-partitions (memory shard transposed host-side to
[512 d, 16384 n]) so the gate value g[d] is a per-partition scalar.

|m-g| split: DVE pieces compute min(m-g,0) via one fused
tensor_scalar(sub, min); the m-term sum rides on the PE (ones^T @ m,
gate-independent) and the g-term is corrected on host per (k, group).
ScalarE pieces compute |m-g| in one op via activation(Abs, scale=-1,
bias=+g). Reductions over d run on the PE into psum rows at quadrant
offsets (4-way tile_position concurrency).

v5 design, driven by traces of 4 prior variants:
 - The single HWDGE stream delivers COMPLETIONS as a FIFO conveyor at
   ~1.25us per DMA nearly independent of size (HBM-receipt pipeline),
   so DMA COUNT is the currency: W ships as ONE 2MB DMA, the 16 memory
   tiles are unsplit, and outputs are 5 DMAs ([4,1024] fp16 per k + the
   gate-sum row). No SWDGE anywhere (it skews SDMA engine 15 ~10%
   slower from t0); no DMA gen on the ScalarE queue (it costs ABS
   throughput); outputs ride the sync ring after the last input gen.
 - The gate runs per 128-d chunk (32 matmuls each, quadrant-cycled;
   fused strip-sum/transpose matmul; Sigmoid straight off psum with
   per-partition bias b), with the PE HAM-warmed by junk matmuls so
   g[c] lands at ~0.9us intervals right behind the single W completion
   - just ahead of the first four tiles' completions. Elementwise
   starts ~5us earlier than with the monolithic gate.
 - psum->SBUF bank copies are fp16 (2x engine rate, half the out-DMA
   bytes); the host upcasts.
"""
import sys

sys.path.insert(0, "/opt/trn_rl_repo")

import numpy as np
import ml_dtypes

import concourse.bacc as bacc
import concourse.mybir as mybir
import concourse.tile as tile
from concourse.bass_utils import run_bass_kernel_spmd

N_CORES = 8
D = 4096
N = 16384
D_SH = D // N_CORES            # 512 dims per core
DC = D_SH // 128               # 4 d-chunks (partition blocks)
NT = 4096                      # n per tile
NK = N // NT                   # 4 n-chunks
NG = NT // 512                 # 8 psum groups per tile
THRESHOLD = 0.8

# ScalarE (Abs) tiles; everything else takes the DVE min path
_ACT_TILES = {(0, 3), (1, 2), (2, 0), (2, 2), (3, 0)}


def _dve_chunks(k, j):
    return [c for c in range(DC) if (k, c) not in _ACT_TILES]


_CACHE = {}


def _build():
    f32 = mybir.dt.float32
    f16 = mybir.dt.float16
    f8 = mybir.dt.float8e3
    A = mybir.AluOpType
    AF = mybir.ActivationFunctionType
    nc = bacc.Bacc(
        "TRN2", target_bir_lowering=False, debug=False, num_devices=N_CORES
    )

    memT = nc.dram_tensor("memT", [D_SH, N], f8, kind="ExternalInput")
    # W shard, host-packed c-major:
    # wtpc[p, c*4096 + j*128 + n'] = W[cglobal*128 + n', j*128 + p]
    wtpc = nc.dram_tensor("wtpc", [128, 32 * D_SH], f8, kind="ExternalInput")
    # packed fp8 constants: cols 0:32 qcol, 32 ones, 33 neg2, 34 selsum
    c8 = nc.dram_tensor("c8", [128, 35], f8, kind="ExternalInput")
    # packed f32 constants: cols 0:4 b columns, 4 ones
    c32 = nc.dram_tensor("c32", [128, 5], f32, kind="ExternalInput")
    # rows 4k+i: group j=4h+i of block k at cols [512h:512h+512]; row 16
    # cols 0:4 = per-chunk gate sums
    outp = nc.dram_tensor("outp", [17, 1024], f16, kind="ExternalOutput")

    with tile.TileContext(nc) as tc:
        with (
            tc.tile_pool(name="wts", bufs=1) as wpool,
            tc.tile_pool(name="mem", bufs=16) as mpool,
            tc.tile_pool(name="dts", bufs=3) as dpool,
            tc.tile_pool(name="acts", bufs=2) as apool,
            tc.tile_pool(name="cp", bufs=4) as cppool,
            tc.tile_pool(name="small", bufs=1) as spool,
            tc.tile_pool(name="psz", bufs=1, space="PSUM") as pzpool,
            tc.tile_pool(name="pst", bufs=1, space="PSUM") as ptpool,
            tc.tile_pool(name="psb", bufs=6, space="PSUM") as pspool,
        ):
            # ---- constants first, then the single W DMA ----
            c8_sb = spool.tile([128, 35], f8, tag="c8")
            nc.sync.dma_start(out=c8_sb[:], in_=c8[:])
            wt_a = wpool.tile([128, 16 * D_SH], f8, tag="wta")
            nc.sync.dma_start(out=wt_a[:], in_=wtpc[:, 0 : 16 * D_SH])
            mt00 = mpool.tile([128, NT], f8, tag="m", name="mt00")
            nc.sync.dma_start(out=mt00[:], in_=memT[0:128, 0:NT])
            wt_b = wpool.tile([128, 16 * D_SH], f8, tag="wtb")
            nc.sync.dma_start(out=wt_b[:], in_=wtpc[:, 16 * D_SH : 32 * D_SH])
            c32_sb = spool.tile([128, 5], f32, tag="c32")
            nc.scalar.dma_start(out=c32_sb[:], in_=c32[:])
            qc_sb = c8_sb[:, 0:32]
            ones_sb = c8_sb[:, 32:33]
            neg2_sb = c8_sb[:, 33:34]
            selsum8 = c8_sb[:, 34:35]
            b4 = c32_sb[:, 0:4]
            ones32_sb = c32_sb[:, 4:5]
            # activation table preload with no DMA dependency
            dum_in = spool.tile([1, 4], f32, tag="dumin")
            nc.vector.memset(dum_in[:], 0.25)
            dum = spool.tile([1, 4], f32, tag="dum")
            nc.scalar.activation(dum[:], dum_in[:], AF.Sigmoid)
            nc.scalar.activation(dum[:], dum_in[:], AF.Abs)

            zps = pzpool.tile([128, D_SH], f32, tag="z")
            nc.vector.memset(zps[:], 0.0)
            ztp = ptpool.tile([128, 512], f32, tag="ztp")
            zcp = spool.tile([128, D_SH], f16, tag="zcp")
            gpos = spool.tile([128, DC], f32, tag="gpos")

            # PE warmup: junk matmuls (zeroed SBUF, psum region reused by
            # strip-sums whose start=True clears it) keep HAM at K=8/8
            # through the W-completion so the gate runs at 2.4GHz.
            wjunk = spool.tile([128, 512], f8, tag="wjunk")
            nc.vector.memset(wjunk[:], 0.5)
            for _w in range(14):
                nc.tensor.matmul(
                    ztp[0:1, :], wjunk[:, 0:1], wjunk[:, 0:512],
                    start=(_w == 0), stop=(_w == 13), skip_group_check=True,
                )

            # ---- gate, per 128-d chunk ----
            for c in range(DC):
                for j in range(32):
                    r = j % 4
                    nc.tensor.matmul(
                        zps[32 * r : 32 * r + 1, c * 128 : (c + 1) * 128],
                        qc_sb[:, j : j + 1],
                        (wt_a if c < 2 else wt_b)[
                            :, (c % 2) * NT + j * 128 : (c % 2) * NT + (j + 1) * 128
                        ],
                        start=(j < 4),
                        stop=(j >= 28),
                        tile_position=(0, 32 * r),
                        skip_group_check=True,
                    )
                with tc.high_priority():
                    csl = slice(c * 128, (c + 1) * 128)
                    nc.scalar.activation(zcp[:, csl], zps[:, csl], AF.Copy)
                    # fused strip-sum + transpose
                    nc.tensor.matmul(
                        ztp[:, c : c + 1],
                        zcp[:, csl],
                        selsum8,
                        start=True,
                        stop=True,
                        skip_group_check=True,
                    )
                    # g_c = sigmoid(z_c + b_c) straight from psum
                    nc.scalar.activation(
                        gpos[:, c : c + 1], ztp[:, c : c + 1], AF.Sigmoid,
                        bias=b4[:, c : c + 1],
                    )

            # gate-sum correction, right after the gate
            with tc.high_priority():
                gsps = ztp[0:1, 4:8]
                nc.tensor.matmul(
                    gsps, ones32_sb, gpos[:], start=True, stop=True,
                    skip_group_check=True,
                )
                gs_sb = spool.tile([1, DC], f16, tag="gs")
                nc.scalar.activation(gs_sb[:], gsps, AF.Copy)

            out_dmas = [(outp[16:17, 0:DC], gs_sb[:])]

            # ---- main loop ----
            for k in range(NK):
                banks = [
                    pspool.tile([128, 512], f32, tag="bank", name=f"bank{k}a"),
                    pspool.tile([128, 512], f32, tag="bank", name=f"bank{k}b"),
                ]
                total = [0] * NG
                for c in range(DC):
                    npass = 1 if (k, c) in _ACT_TILES else 2
                    for j in range(NG):
                        total[j] += npass
                seen = [0] * NG
                mts = []
                for c in range(DC):
                    if k == 0 and c == 0:
                        mt = mt00
                    else:
                        mt = mpool.tile([128, NT], f8, tag="m")
                        nc.sync.dma_start(
                            out=mt[:],
                            in_=memT[c * 128 : (c + 1) * 128, k * NT : (k + 1) * NT],
                        )
                    mts.append(mt)
                    if (k, c) not in _ACT_TILES:
                        for j in range(NG):
                            nc.tensor.matmul(
                                banks[j // 4][32 * (j % 4) : 32 * (j % 4) + 1, :],
                                ones_sb,
                                mt[:, j * 512 : (j + 1) * 512],
                                start=(seen[j] == 0),
                                stop=(seen[j] == total[j] - 1),
                                tile_position=(0, 32 * (j % 4)),
                                skip_group_check=True,
                            )
                            seen[j] += 1
                for c in range(DC):
                    mt = mts[c]
                    if (k, c) in _ACT_TILES:
                        at = apool.tile([128, NT], f8, tag="a")
                        nc.scalar.activation(
                            at[:], mt[:], AF.Abs,
                            bias=gpos[:, c : c + 1], scale=-1.0,
                        )
                        src_, stat = at, ones_sb
                    else:
                        dt = dpool.tile([128, NT], f8, tag="d")
                        nc.vector.tensor_scalar(
                            dt[:], mt[:],
                            gpos[:, c : c + 1], 0.0,
                            A.subtract, A.min,
                        )
                        src_, stat = dt, neg2_sb
                    for j in range(NG):
                        nc.tensor.matmul(
                            banks[j // 4][32 * (j % 4) : 32 * (j % 4) + 1, :],
                            stat,
                            src_[:, j * 512 : (j + 1) * 512],
                            start=(seen[j] == 0),
                            stop=(seen[j] == total[j] - 1),
                            tile_position=(0, 32 * (j % 4)),
                            skip_group_check=True,
                        )
                        seen[j] += 1
                cp = cppool.tile([128, 1024], f16, tag="cp")
                for h in range(2):
                    csl = slice(512 * h, 512 * h + 512)
                    if k == 3 and h == 1:
                        nc.vector.tensor_copy(cp[:, csl], banks[h][:])
                    else:
                        nc.scalar.activation(cp[:, csl], banks[h][:], AF.Copy)
                out_dmas.append((outp[4 * k : 4 * k + 4, :], cp[0:128:32, :]))

            # ---- outputs: sync ring, after every input dma_start ----
            for dst, src in out_dmas:
                nc.sync.dma_start(out=dst, in_=src)

    nc.compile()
    return nc


def _get_nc():
    if "nc" not in _CACHE:
        _CACHE["nc"] = _build()
    return _CACHE["nc"]


def kernel(query, W, b, memory, _trace=False, _return_raw=False):
    f8 = ml_dtypes.float8_e3m4
    query = np.asarray(query, dtype=np.float32)
    W = np.asarray(W, dtype=np.float32)
    b = np.asarray(b, dtype=np.float32)
    memory = np.asarray(memory, dtype=np.float32)

    mem8T = np.ascontiguousarray(memory.astype(f8).T)       # [D, N] fp8
    W8 = W.astype(f8)
    q8 = query.reshape(32, 128).astype(f8).T                # [128, 32]
    c8 = np.zeros((128, 35), dtype=f8)
    c8[:, 0:32] = q8
    c8[:, 32] = f8(1.0)
    c8[:, 33] = f8(-2.0)
    c8[0:128:32, 34] = f8(1.0)

    in_maps = []
    for c in range(N_CORES):
        sl = slice(c * D_SH, (c + 1) * D_SH)
        wsh = W8[sl, :]                       # [512, 4096]
        # wtpc[p, cc*4096 + j*128 + n'] = wsh[cc*128 + n', j*128 + p]
        wtpc = np.ascontiguousarray(
            wsh.reshape(4, 128, 32, 128).transpose(3, 0, 2, 1).reshape(128, -1)
        )
        c32 = np.zeros((128, 5), dtype=np.float32)
        c32[:, 0:4] = b[sl].reshape(4, 128).T
        c32[:, 4] = 1.0
        in_maps.append(
            {
                "memT": np.ascontiguousarray(mem8T[sl, :]),
                "wtpc": wtpc,
                "c8": c8,
                "c32": c32,
            }
        )

    nc = _get_nc()
    res = run_bass_kernel_spmd(
        nc, in_maps, list(range(N_CORES)), trace=_trace
    )

    total = np.zeros(N, dtype=np.float64)
    for c in range(N_CORES):
        out = res.results[c]["outp"].astype(np.float64)
        gsum = out[16, 0:DC]                  # sum of g per d-chunk
        # row 4k+i, col 512h+n  ->  block k, group j=4h+i
        rows = out[0:16].reshape(NK, 4, 2, 512).transpose(0, 2, 1, 3)
        rows = np.ascontiguousarray(rows).reshape(NK, NG, 512)
        corr = np.array(
            [
                [sum(gsum[ci] for ci in _dve_chunks(k, j)) for j in range(NG)]
                for k in range(NK)
            ]
        )
        total += (rows - corr[:, :, None]).reshape(N)
    sims = (1.0 - total / D).astype(np.float32)
    mask = sims >= THRESHOLD
    if _return_raw:
        return (sims, mask), res
    return sims, mask
